# revision 1
# baseline (speedup 1.0000x reference)
"""ColorCurveLearningLoss on 8 Trainium2 NeuronCores (measured 182 us/core).

Math: pred_curve - target_curve = sum_bin(pred - target) / count, so the
kernel only needs per-(channel,bin) sums of d = pred - target and counts;
the 8 cores' partials combine by addition on the host, followed by the
division and L1 mean (96 values -- negligible).

Device pipeline per core (data-parallel over batch, 2 images/core):
  - ScalarE: exact floor(32x): w = relu(32x - 2^-19) + 32 puts floor(32x)
    in mantissa bits 18..22 of w (half-ulp-down bias makes round-to-nearest
    land every fp32 value in the correct bin; relu clamps x < 2^-24, incl.
    0, into bin 0).
  - VectorE: bitvec extract idx = (w >> 18) & 31 (int32, in place), then
    32 onehot blocks th[:, j, :] = (idx == j) in bf16 4x mode (the
    throughput wall, at the DVE write-bandwidth floor), plus
    d = pred - target into the interleaved dn = [d|1] tile.
  - ScalarE: idx int32 -> bf16 convert, the dn ones column, and the PSUM
    drains (keeps them off the DVE).
  - TensorE: per 16-chunk group, matmul(lhsT=dn[128,16,2] -> M=32,
    rhs=th[128,32,16] -> N=512) accumulates [sum_d; count] into PSUM;
    groups round-robin over 4 array col-groups (tile_position) in 4
    separate PSUM banks; banks ping-pong across channels (8 banks total)
    so a channel's first matmuls never wait on the previous drain.
  - GPSIMD: idle (its elementwise ops contend with DVE for SBUF ports;
    measured 6x slowdown when used).

d is quantized to bf16 (sums accumulate in fp32 PSUM): final scalar rel
err ~5e-4.  Bin counts are exact integers (verified elementwise vs numpy).
Engine occupancy: VectorE ~155 us (bound), DMA ~83 us (~310 GB/s),
TensorE ~70 us, ScalarE ~65 us.
"""

import numpy as np

NB = 32
B, C, H, W = 16, 3, 512, 512
N_CORES = 8
B_PER_CORE = B // N_CORES
ELEMS_PER_CH = B_PER_CORE * H * W  # 524288
P = 128
F = 1024
SUPER_F = 1024
N_SUPER = ELEMS_PER_CH // (P * SUPER_F)  # 2
N_SUB = SUPER_F // F  # 4
GROUP = 16
NCOLG = 4
N_GP_BLOCKS = 0  # GPSIMD elementwise ops contend for SBUF ports with DVE

_CACHE = {}
_HALF_ULP = float(2.0 ** -19)


def _build(nch=C, n_super=N_SUPER):
    import concourse.bass as bass
    import concourse.tile as tile
    from concourse import bacc, mybir

    nc = bacc.Bacc("TRN2", target_bir_lowering=False, debug=False,
                   num_devices=N_CORES)
    f32 = mybir.dt.float32
    bf16 = mybir.dt.bfloat16
    Relu = mybir.ActivationFunctionType.Relu
    Identity = mybir.ActivationFunctionType.Identity
    Copy = mybir.ActivationFunctionType.Copy

    # activation biases must exist as const APs
    for val in (-_HALF_ULP, float(NB)):
        t = nc.alloc_sbuf_tensor(f"constx-{val}", [128, 1], f32)
        nc.gpsimd.memset(t.ap(), val)
        nc.const_aps.aps[(f32, val)] = t.ap()
    nc.all_engine_barrier()

    xin = nc.dram_tensor("xin", [nch, n_super, P, SUPER_F], f32,
                         kind="ExternalInput")
    pin = nc.dram_tensor("pin", [nch, n_super, P, SUPER_F], f32,
                         kind="ExternalInput")
    tin = nc.dram_tensor("tin", [nch, n_super, P, SUPER_F], f32,
                         kind="ExternalInput")
    out = nc.dram_tensor("out", [P, nch * NB * GROUP], f32,
                         kind="ExternalOutput")

    n_groups_per_sub = F // GROUP  # 32
    n_mm_per_cg = n_super * N_SUB * n_groups_per_sub // NCOLG

    with tile.TileContext(nc) as tc:
        with (
            tc.tile_pool(name="inp", bufs=3) as inp,
            tc.tile_pool(name="work", bufs=2) as work,
            tc.tile_pool(name="acc", bufs=1) as accp,
            tc.tile_pool(name="ps", bufs=1, space="PSUM") as ps,
        ):
            psum_all = ps.tile([P, 2, NCOLG, NB * GROUP], f32,
                               tag="psum_all", name="psum_all")
            res = accp.tile([P, nch * NB * GROUP], f32, name="res")

            for c in range(nch):
                mm_cg = [0] * NCOLG
                for sb in range(n_super):
                    xt = inp.tile([P, SUPER_F], f32, tag="x", name="xt")
                    nc.sync.dma_start(out=xt[:], in_=xin[c, sb])
                    pt_in = inp.tile([P, SUPER_F], f32, tag="p", name="pt_in")
                    nc.sync.dma_start(out=pt_in[:], in_=pin[c, sb])
                    tt_in = inp.tile([P, SUPER_F], f32, tag="t", name="tt_in")
                    nc.sync.dma_start(out=tt_in[:], in_=tin[c, sb])

                    for sub in range(N_SUB):
                        xs = xt[:, sub * F:(sub + 1) * F]
                        # ScalarE floor pipeline (see v2 notes):
                        # w = relu(32x - 2^-19) + 32  => exact floor(32x) in
                        # mantissa bits 18..22; x < 2^-24 lands in bin 0.
                        w = work.tile([P, F], f32, tag="w", name="w")
                        nc.scalar.activation(
                            out=w[:], in_=xs, func=Relu,
                            bias=-_HALF_ULP, scale=float(NB))
                        nc.scalar.activation(
                            out=w[:], in_=w[:], func=Identity,
                            bias=float(NB), scale=1.0)
                        wi = w[:].bitcast(mybir.dt.int32)
                        nc.vector.tensor_scalar(
                            out=wi, in0=wi, scalar1=18, scalar2=NB - 1,
                            op0=mybir.AluOpType.logical_shift_right,
                            op1=mybir.AluOpType.bitwise_and)
                        idxb = work.tile([P, F], bf16, tag="idxb", name="idxb")
                        nc.scalar.activation(out=idxb[:], in_=wi, func=Copy)
                        th = work.tile([P, NB, F], bf16, tag="th", name="th")
                        for j in range(NB):
                            nc.vector.tensor_scalar(
                                out=th[:, j, :], in0=idxb[:],
                                scalar1=float(j), scalar2=None,
                                op0=mybir.AluOpType.is_equal)
                        dn = work.tile([P, F, 2], bf16, tag="dn", name="dn")
                        nc.scalar.activation(
                            out=dn[:, :, 1], in_=xs, func=Identity,
                            bias=1.0, scale=0.0)
                        nc.vector.tensor_tensor(
                            out=dn[:, :, 0],
                            in0=pt_in[:, sub * F:(sub + 1) * F],
                            in1=tt_in[:, sub * F:(sub + 1) * F],
                            op=mybir.AluOpType.subtract)
                        for gi in range(n_groups_per_sub):
                            f0 = gi * GROUP
                            gc = gi % NCOLG
                            nc.tensor.matmul(
                                psum_all[32 * gc:32 * (gc + 1), c % 2, gc, :],
                                lhsT=dn[:, f0:f0 + GROUP, :],
                                rhs=th[:, :, f0:f0 + GROUP],
                                start=(mm_cg[gc] == 0),
                                stop=(mm_cg[gc] == n_mm_per_cg - 1),
                                tile_position=(0, 32 * gc),
                            )
                            mm_cg[gc] += 1

                # drain this channel's psum banks to res rows per col-group
                for gc in range(NCOLG):
                    nc.scalar.copy(
                        out=res[32 * gc:32 * (gc + 1),
                                c * NB * GROUP:(c + 1) * NB * GROUP],
                        in_=psum_all[32 * gc:32 * (gc + 1), c % 2, gc, :])

            nc.sync.dma_start(out=out[:], in_=res[:])

    nc.compile()
    return nc


def _get_nc():
    if "nc" not in _CACHE:
        _CACHE["nc"] = _build()
    return _CACHE["nc"]


def _shard(arr, core):
    a = arr[core * B_PER_CORE:(core + 1) * B_PER_CORE]
    a = np.ascontiguousarray(np.transpose(a, (1, 0, 2, 3)))
    return a.reshape(C, N_SUPER, P, SUPER_F).astype(np.float32, copy=False)


def _decode(raw, nch=C):
    """raw [P, nch*NB*GROUP]; rows 32gc..32gc+31 hold col-group gc."""
    S = np.zeros((nch, NB), np.float64)
    Cnt = np.zeros((nch, NB), np.float64)
    for c in range(nch):
        slab = raw[:, c * NB * GROUP:(c + 1) * NB * GROUP]
        r = slab.reshape(NCOLG, GROUP, 2, NB, GROUP)
        for g in range(GROUP):
            S[c] += r[:, g, 0, :, g].sum(axis=0)
            Cnt[c] += r[:, g, 1, :, g].sum(axis=0)
    return S, Cnt


def _finalize(S, Cnt):
    diff = np.where(Cnt > 0, np.abs(S) / np.maximum(Cnt, 1.0), 0.0)
    return np.float32(diff.mean())


def kernel(pred, target, input_img):
    from concourse.bass_utils import run_bass_kernel_spmd

    nc = _get_nc()
    in_maps = []
    for core in range(N_CORES):
        in_maps.append({
            "xin": _shard(np.asarray(input_img), core),
            "pin": _shard(np.asarray(pred), core),
            "tin": _shard(np.asarray(target), core),
        })
    res = run_bass_kernel_spmd(nc, in_maps, list(range(N_CORES)))
    S = np.zeros((C, NB), np.float64)
    Cnt = np.zeros((C, NB), np.float64)
    for r in res.results:
        s, cc = _decode(r["out"])
        S += s
        Cnt += cc
    _CACHE["last_SC"] = (S, Cnt)
    return np.asarray(_finalize(S, Cnt), dtype=np.float32)



# revision 2
# speedup vs baseline: 1.0137x; 1.0137x over previous
"""ColorCurveLearningLoss on 8 Trainium2 NeuronCores — factorized-basis v2.

Math: only per-(channel,bin) sums of d = pred - target and counts are
needed; cores' partials combine on the host (division + L1 mean on 96
values is negligible).

Key change vs v1 (181.7 us): v1 materialized a full 32-plane one-hot on
the DVE (the measured bottleneck at ~155 us busy).  v2 factorizes the
bin index idx = 4*hi + lo (hi = idx>>2 in [0,8), lo = idx&3) and lets
the TensorE matmul form the product [lo-basis] x [hi-basis]:

  out[r, s] = sum_p m_r[p] * n_s[p]
  m_r in { d, d*[lo>=1], d*[lo>=2], d*[lo>=3],       (M side, lhsT)
           1,   [lo>=1],   [lo>=2],   [lo>=3] }
  n_s = [hi >= s], s = 0..7                          (N side, rhs)

Thermometer bases are single fused tensor_scalar ops (4x DVE mode), and
the 8x8 / 4x4 basis matrices have exact integer inverses applied on the
host, so bin counts stay exact.  DVE planes per tile: 12 ts + 4 tt vs
v1's 32 ts + extras -> predicted DVE ~86 us, TensorE ~41 us (768
matmuls of N=128), ScalarE ~50 us, DMA ~83 us.

Per sub-tile [128, 1024] pipeline:
  ScalarE: w = relu(32x - 2^-19) + 32 (exact floor(32x) in mantissa
    bits 18..22, x=0 lands in bin 0 — v1's proven construction), plus
    bf16 downconverts of pred/target.
  DVE: idx5 = (w>>18)&31 -> bf16; 12 thermometer planes via fused
    tensor_scalar(mod/is_ge); d = p - t and 3 d*[lo>=r] tensor_tensor.
  TensorE: per 16-column group, matmul(lhsT=mm[128,8,16] -> M=128,
    rhs=hi[128,8,16] -> N=128) accumulates into one [128,128] PSUM
    region per channel (256-matmul chains).
  Host: diagonal-extract f==f', sum over f, invert thermometer bases,
    fold cores, then divide + L1 mean.
"""

import numpy as np

NB = 32
B, C, H, W = 16, 3, 512, 512
N_CORES = 8
B_PER_CORE = B // N_CORES
ELEMS_PER_CH = B_PER_CORE * H * W  # 524288
P = 128
F = 1024
N_SUPER = ELEMS_PER_CH // (P * F)  # 4
GROUP = 16
NGROUPS = F // GROUP  # 64
NLO = 4   # lo levels (2 bits)
NHI = 8   # hi levels (3 bits)

_CACHE = {}
_HALF_ULP = float(2.0 ** -19)


def _build():
    import concourse.bass as bass
    import concourse.tile as tile
    from concourse import bacc, mybir

    nc = bacc.Bacc("TRN2", target_bir_lowering=False, debug=False,
                   num_devices=N_CORES)
    f32 = mybir.dt.float32
    bf16 = mybir.dt.bfloat16
    Relu = mybir.ActivationFunctionType.Relu
    Identity = mybir.ActivationFunctionType.Identity
    Copy = mybir.ActivationFunctionType.Copy
    Alu = mybir.AluOpType

    # activation biases must exist as const APs
    for val in (-_HALF_ULP, float(NB)):
        t = nc.alloc_sbuf_tensor(f"constx-{val}", [128, 1], f32)
        nc.gpsimd.memset(t.ap(), val)
        nc.const_aps.aps[(f32, val)] = t.ap()
    nc.all_engine_barrier()

    xin = nc.dram_tensor("xin", [C, N_SUPER, P, NGROUPS, GROUP], f32,
                         kind="ExternalInput")
    pin = nc.dram_tensor("pin", [C, N_SUPER, P, NGROUPS, GROUP], f32,
                         kind="ExternalInput")
    tin = nc.dram_tensor("tin", [C, N_SUPER, P, NGROUPS, GROUP], f32,
                         kind="ExternalInput")
    out = nc.dram_tensor("out", [P, C * NHI * GROUP], f32,
                         kind="ExternalOutput")

    n_mm_per_ch = N_SUPER * NGROUPS  # 256

    with tile.TileContext(nc) as tc:
        with (
            tc.tile_pool(name="inp", bufs=3) as inp,
            tc.tile_pool(name="work", bufs=2) as work,
            tc.tile_pool(name="acc", bufs=1) as accp,
            tc.tile_pool(name="ps", bufs=1, space="PSUM") as ps,
        ):
            # one PSUM bank (512 f32) per channel; region [:, c, 0:128] used
            psum_all = ps.tile([P, C, 512], f32, tag="psum_all",
                               name="psum_all")
            res = accp.tile([P, C * NHI * GROUP], f32, name="res")

            for c in range(C):
                mm_i = 0
                for sb in range(N_SUPER):
                    xt = inp.tile([P, NGROUPS, GROUP], f32, tag="x",
                                  name="xt")
                    nc.sync.dma_start(out=xt[:], in_=xin[c, sb])
                    pt_in = inp.tile([P, NGROUPS, GROUP], f32, tag="p",
                                     name="pt_in")
                    nc.sync.dma_start(out=pt_in[:], in_=pin[c, sb])
                    tt_in = inp.tile([P, NGROUPS, GROUP], f32, tag="t",
                                     name="tt_in")
                    nc.sync.dma_start(out=tt_in[:], in_=tin[c, sb])

                    # ScalarE floor pipeline (v1's exact construction):
                    # w = relu(32x - 2^-19) + 32 => floor(32x) in mantissa
                    # bits 18..22; x < 2^-24 (incl. 0) lands in bin 0.
                    w = work.tile([P, NGROUPS, GROUP], f32, tag="w",
                                  name="w")
                    nc.scalar.activation(out=w[:], in_=xt[:], func=Relu,
                                         bias=-_HALF_ULP, scale=float(NB))
                    nc.scalar.activation(out=w[:], in_=w[:], func=Identity,
                                         bias=float(NB), scale=1.0)
                    # lo = (w >> 18) & 3 to a fresh tile, then
                    # idx5 = (w >> 18) & 31 in place (bitvec ops can't
                    # cast, and mod isn't in the DVE ISA, so the periodic
                    # lo needs its own extract); int32 -> bf16 on ScalarE
                    wi = w[:].bitcast(mybir.dt.int32)
                    lo_t = work.tile([P, NGROUPS, GROUP], f32, tag="lot",
                                     name="lo_t")
                    nc.vector.tensor_scalar(
                        out=lo_t[:].bitcast(mybir.dt.int32), in0=wi,
                        scalar1=18, scalar2=3,
                        op0=Alu.logical_shift_right, op1=Alu.bitwise_and)
                    nc.vector.tensor_scalar(
                        out=wi, in0=wi, scalar1=18, scalar2=NB - 1,
                        op0=Alu.logical_shift_right, op1=Alu.bitwise_and)
                    idx5 = work.tile([P, NGROUPS, GROUP], bf16, tag="idx5",
                                     name="idx5")
                    nc.scalar.activation(out=idx5[:], in_=wi, func=Copy)
                    lo_b = work.tile([P, NGROUPS, GROUP], bf16, tag="lob",
                                     name="lo_b")
                    nc.scalar.activation(
                        out=lo_b[:], in_=lo_t[:].bitcast(mybir.dt.int32),
                        func=Copy)

                    # bf16 copies of pred/target (ScalarE) for 2x-mode DVE sub
                    pb = work.tile([P, NGROUPS, GROUP], bf16, tag="pb",
                                   name="pb")
                    nc.scalar.activation(out=pb[:], in_=pt_in[:], func=Copy)
                    tb = work.tile([P, NGROUPS, GROUP], bf16, tag="tb",
                                   name="tb")
                    nc.scalar.activation(out=tb[:], in_=tt_in[:], func=Copy)

                    # N side: hi thermometer [hi>=s] == [idx5 >= 4s];
                    # group-blocked layout so each matmul operand block
                    # [8, GROUP] is contiguous (BIR needs one free dim)
                    hi = work.tile([P, NGROUPS, NHI, GROUP], bf16, tag="hi",
                                   name="hi")
                    for s in range(NHI):
                        nc.vector.tensor_scalar(
                            out=hi[:, :, s, :], in0=idx5[:],
                            scalar1=float(4 * s), scalar2=None,
                            op0=Alu.is_ge)

                    # M side planes: [d, d*t1, d*t2, d*t3, 1, t1, t2, t3]
                    # with tr = [lo >= r]
                    mm = work.tile([P, NGROUPS, 2 * NLO, GROUP], bf16,
                                   tag="mm", name="mm")
                    for r in range(NLO):
                        nc.vector.tensor_scalar(
                            out=mm[:, :, 4 + r, :], in0=lo_b[:],
                            scalar1=float(r), scalar2=None, op0=Alu.is_ge)
                    nc.vector.tensor_tensor(
                        out=mm[:, :, 0, :], in0=pb[:], in1=tb[:],
                        op=Alu.subtract)
                    for r in (1, 2, 3):
                        nc.vector.tensor_tensor(
                            out=mm[:, :, r, :], in0=mm[:, :, 0, :],
                            in1=mm[:, :, 4 + r, :], op=Alu.mult)

                    for gi in range(NGROUPS):
                        nc.tensor.matmul(
                            psum_all[:, c, 0:NHI * GROUP],
                            lhsT=mm[:, gi],
                            rhs=hi[:, gi],
                            start=(mm_i == 0),
                            stop=(mm_i == n_mm_per_ch - 1),
                        )
                        mm_i += 1

                nc.scalar.copy(
                    out=res[:, c * NHI * GROUP:(c + 1) * NHI * GROUP],
                    in_=psum_all[:, c, 0:NHI * GROUP])

            nc.sync.dma_start(out=out[:], in_=res[:])

    nc.compile()
    return nc


def _get_nc():
    if "nc" not in _CACHE:
        _CACHE["nc"] = _build()
    return _CACHE["nc"]


def _shard(arr, core):
    a = arr[core * B_PER_CORE:(core + 1) * B_PER_CORE]
    a = np.ascontiguousarray(np.transpose(a, (1, 0, 2, 3)))
    return a.reshape(C, N_SUPER, P, NGROUPS, GROUP).astype(
        np.float32, copy=False)


# thermometer basis matrices (exact integer inverses)
_PHI = np.array([[1 if l >= r else 0 for l in range(NLO)]
                 for r in range(NLO)], np.float64)  # Phi[r, l] = [l >= r]
_PSI = np.array([[1 if h >= s else 0 for h in range(NHI)]
                 for s in range(NHI)], np.float64)  # Psi[s, h] = [h >= s]
_PHI_INV = np.linalg.inv(_PHI)
_PSI_INV = np.linalg.inv(_PSI)


def _decode(raw):
    """raw [P, C*128]: per channel, PSUM block rows m=r*16+f, cols
    n=s*16+f'; valid entries f==f'. Returns (S[C,NB], Cnt[C,NB])."""
    S = np.zeros((C, NB), np.float64)
    Cnt = np.zeros((C, NB), np.float64)
    for c in range(C):
        blk = raw[:, c * 128:(c + 1) * 128].astype(np.float64)
        q = blk.reshape(2 * NLO, GROUP, NHI, GROUP)
        o2 = np.einsum('rfsf->rs', q)  # diag f==f', summed over f
        Bd = _PHI_INV @ o2[0:NLO, :] @ _PSI_INV.T   # [lo, hi]
        Bc = _PHI_INV @ o2[NLO:2 * NLO, :] @ _PSI_INV.T
        S[c] = Bd.T.reshape(-1)    # bin = 4*hi + lo
        Cnt[c] = Bc.T.reshape(-1)
    return S, Cnt


def _finalize(S, Cnt):
    diff = np.where(Cnt > 0, np.abs(S) / np.maximum(Cnt, 1.0), 0.0)
    return np.float32(diff.mean())


def kernel(pred, target, input_img):
    from concourse.bass_utils import run_bass_kernel_spmd

    nc = _get_nc()
    in_maps = []
    for core in range(N_CORES):
        in_maps.append({
            "xin": _shard(np.asarray(input_img), core),
            "pin": _shard(np.asarray(pred), core),
            "tin": _shard(np.asarray(target), core),
        })
    res = run_bass_kernel_spmd(nc, in_maps, list(range(N_CORES)))
    S = np.zeros((C, NB), np.float64)
    Cnt = np.zeros((C, NB), np.float64)
    for r in res.results:
        s, cc = _decode(r["out"])
        S += s
        Cnt += cc
    _CACHE["last_SC"] = (S, Cnt)
    return np.asarray(_finalize(S, Cnt), dtype=np.float32)
